# revision 8
# baseline (speedup 1.0000x reference)
"""Trainium2 Bass kernel v2 for nn_CrossAttn (linear cross-attention, B=8 N=4096 C=1024 H=16).

Strategy (per core, one batch element; data-parallel over B across 8 cores):
  - All activations transposed [C, N] bf16 (C on partitions); weights bf16.
  - Streaming slab design: x tensors move through a 16-slot rotation pool of
    [128, KT, 512] slabs; x1' spills to DRAM during S1c and streams back in
    during the fused cross phase; x2' stays resident.
  - ctx accumulation lives entirely in PSUM: 8 head-pairs packed 4-per-bank in
    2 banks; pair p%4==0 issues start=True (clears the bank's has_written),
    the other pairs' first matmuls rely on overwrite-where-cleared; all later
    n-tiles accumulate.  No DVE adds.
  - PSUM->SBUF copies alternate between DVE and ACT to halve the copy wall.
  - Softmax reads ctx straight from PSUM; S is bf16; PE transpose in bf16.
  - Phases: S1a(ctx1) sm1 S1c(x1'=q@ctx+x, spill) | S2a sm2 S2c(x2' resident)
    | C2a(ctx2 from x2') sm | fused{o1 out + ctx1' accum over streamed x1'}
    | sm | o2 out from x2'.
"""

import os
import sys

sys.path.insert(0, "/opt/trn_rl_repo")

import numpy as np
import ml_dtypes

import concourse.bass as bass
import concourse.mybir as mybir
import concourse.tile as tile
from concourse import bacc
from concourse.masks import make_identity
from concourse.bass_utils import run_bass_kernel_spmd

B, N, C, H = 8, 4096, 1024, 16
D = C // H                 # 64
SCALE = D ** -0.5          # 0.125
P = 128
KT = C // P                # 8 contraction tiles of 128
NT = N // P                # 32 n-tiles (ctx accumulation)
CH = N // 512              # 8 n-slabs of 512
PAIRS = H // 2             # 8 head pairs
F32 = mybir.dt.float32
BF16 = mybir.dt.bfloat16
Copy = mybir.ActivationFunctionType.Copy
Exp = mybir.ActivationFunctionType.Exp

_CACHE = {}


def _build():
    nc = bacc.Bacc(None, target_bir_lowering=False)

    x1T_d = nc.dram_tensor("x1T", [C, N], BF16, kind="ExternalInput")
    x2T_d = nc.dram_tensor("x2T", [C, N], BF16, kind="ExternalInput")
    Wsqkv_d = nc.dram_tensor("Wsqkv", [C, 3 * C], BF16, kind="ExternalInput")
    Wkv1_d = nc.dram_tensor("Wkv1", [C, 2 * C], BF16, kind="ExternalInput")
    Wkv2_d = nc.dram_tensor("Wkv2", [C, 2 * C], BF16, kind="ExternalInput")
    o1T_d = nc.dram_tensor("o1T", [C, N], BF16, kind="ExternalOutput")
    o2T_d = nc.dram_tensor("o2T", [C, N], BF16, kind="ExternalOutput")
    x1p_scr = nc.dram_tensor("x1p_scratch", [C, N], BF16, kind="Internal")

    x1T_r = x1T_d[:].rearrange("(t p) n -> p t n", p=P)
    x2T_r = x2T_d[:].rearrange("(t p) n -> p t n", p=P)
    Wsq_r = Wsqkv_d[:].rearrange("(t p) c -> p t c", p=P)
    Wkv1_r = Wkv1_d[:].rearrange("(t p) c -> p t c", p=P)
    Wkv2_r = Wkv2_d[:].rearrange("(t p) c -> p t c", p=P)
    o1T_r = o1T_d[:].rearrange("(t p) n -> p t n", p=P)
    o2T_r = o2T_d[:].rearrange("(t p) n -> p t n", p=P)
    x1p_r = x1p_scr[:].rearrange("(t p) n -> p t n", p=P)

    with tile.TileContext(nc) as tc:
        with (
            tc.tile_pool(name="xrot", bufs=16) as xrot,
            tc.tile_pool(name="wts", bufs=1) as wts,
            tc.tile_pool(name="kvsb", bufs=2) as kvsb,
            tc.tile_pool(name="qts", bufs=8) as qtsp,
            tc.tile_pool(name="ctxsb", bufs=2) as ctxsb,
            tc.tile_pool(name="smax", bufs=8) as smaxp,
            tc.tile_pool(name="stats", bufs=8) as stats,
            tc.tile_pool(name="outst", bufs=2) as outst,
            tc.tile_pool(name="singles", bufs=1) as singles,
            tc.tile_pool(name="ps_kv", bufs=4, space="PSUM") as ps_kv,
            tc.tile_pool(name="ps_ctx", bufs=1, space="PSUM") as ps_ctx,
            tc.tile_pool(name="ps_out", bufs=2, space="PSUM") as ps_out,
        ):
            ident = singles.tile([P, P], BF16)
            make_identity(nc, ident)

            copy_flip = [0]

            def psum_copy(dst, src):
                """PSUM->SBUF copy, alternating DVE / ACT."""
                copy_flip[0] ^= 1
                if copy_flip[0]:
                    nc.vector.tensor_copy(dst, src)
                else:
                    nc.scalar.activation(dst, src, Copy)

            def load_x_slab(x_r, s, who):
                sl = xrot.tile([P, KT, 512], BF16, tag="xsl", name=f"{who}_{s}")
                nc.sync.dma_start(out=sl, in_=x_r[:, :, s * 512:(s + 1) * 512])
                return sl

            def kv_nt(slabs, wk, wv, nt):
                """kv for n-tile nt: [128 n, 2048] bf16 (k cols 0:1024, v 1024:2048)."""
                sl = slabs[nt // 4]
                nsl = slice((nt % 4) * P, (nt % 4) * P + P)
                kv = kvsb.tile([P, 2 * C], BF16, tag="kv")
                for half, w in ((0, wk), (1, wv)):
                    for ch2 in range(2):
                        kv_ps = ps_kv.tile([P, 512], F32, tag="gp")
                        for kt in range(KT):
                            nc.tensor.matmul(
                                kv_ps,
                                lhsT=sl[:, kt, nsl],
                                rhs=w[:, kt, ch2 * 512:(ch2 + 1) * 512],
                                start=(kt == 0), stop=(kt == KT - 1),
                            )
                        psum_copy(kv[:, half * C + ch2 * 512: half * C + (ch2 + 1) * 512],
                                  kv_ps)
                return kv

            def ctx_banks():
                bA = ps_ctx.tile([P, 512], F32, tag="ctxA")
                bB = ps_ctx.tile([P, 512], F32, tag="ctxB")
                return (bA, bB)

            def pair_mms(banks, kv, nt):
                """Accumulate ctx_rawT (rows=e(v-dim), cols=d(k-dim)) for all 8
                pairs into 2 PSUM banks, 4 pairs per bank."""
                for p in range(PAIRS):
                    bank = banks[p // 4]
                    csl = slice((p % 4) * P, (p % 4) * P + P)
                    nc.tensor.matmul(
                        bank[:, csl],
                        lhsT=kv[:, C + p * P: C + (p + 1) * P],   # v pair
                        rhs=kv[:, p * P:(p + 1) * P],             # k pair
                        start=(nt == 0 and p % 4 == 0),
                        stop=(nt == NT - 1),
                        skip_group_check=True,
                    )

            def ctx_accumulate(slabs, wk, wv):
                banks = ctx_banks()
                for nt in range(NT):
                    kv = kv_nt(slabs, wk, wv, nt)
                    pair_mms(banks, kv, nt)
                return banks

            def softmax_chain(banks, p):
                """DVE/ACT softmax chain for pair p; returns the normalized
                bf16 block-diag S tile (rows=e, cols=d)."""
                bank = banks[p // 4]
                c0 = (p % 4) * P
                S = smaxp.tile([P, P], BF16, tag="smax")
                nc.vector.memset(S, 0.0)
                for r0 in (0, 64):
                    blk = bank[r0:r0 + 64, c0 + r0: c0 + r0 + 64]
                    mx = stats.tile([P, 1], F32, tag="mx")
                    nc.vector.reduce_max(mx[r0:r0 + 64], blk, axis=mybir.AxisListType.X)
                    ng = stats.tile([P, 1], F32, tag="ng")
                    nc.scalar.mul(ng[r0:r0 + 64], mx[r0:r0 + 64], -SCALE)
                    se = stats.tile([P, 1], F32, tag="se")
                    nc.scalar.activation(
                        S[r0:r0 + 64, r0:r0 + 64], blk, Exp,
                        bias=ng[r0:r0 + 64], scale=SCALE,
                        accum_out=se[r0:r0 + 64],
                    )
                    rv = stats.tile([P, 1], F32, tag="rv")
                    nc.vector.reciprocal(rv[r0:r0 + 64], se[r0:r0 + 64])
                    nc.vector.tensor_scalar_mul(
                        S[r0:r0 + 64, r0:r0 + 64], S[r0:r0 + 64, r0:r0 + 64],
                        rv[r0:r0 + 64],
                    )
                return S

            def softmax_all(banks):
                """Softmax of all pairs: issue every DVE/ACT chain before the
                first PE transpose so transposes don't stall the PE queue."""
                ctx_bd = ctxsb.tile([P, PAIRS, P], BF16, tag="ctx_bd")
                Ss = [softmax_chain(banks, p) for p in range(PAIRS)]
                for p in range(PAIRS):
                    tr_ps = ps_out.tile([P, P], BF16, tag="psout")
                    nc.tensor.transpose(tr_ps, Ss[p], ident)
                    nc.vector.tensor_copy(ctx_bd[:, p, :], tr_ps)
                return ctx_bd

            def self_out_chunk(xsl, wq, ctx_bd, ch, xp_sink, spill_to=None):
                """One 512-col chunk of x' = q@ctx + x.  xsl: the x slab for
                chunk ch; writes x' into a fresh xrot slab; optionally spills."""
                xp = xrot.tile([P, KT, 512], BF16, tag="xsl", name=f"xp_{ch}")
                # q GEMMs for all pairs first, out-products after: decouples
                # the ctx-dependent MMs so softmax latency hides behind q work.
                qts_all = []
                for p in range(PAIRS):
                    qt_ps = ps_kv.tile([P, 512], F32, tag="gp")
                    for kt in range(KT):
                        nc.tensor.matmul(
                            qt_ps,
                            lhsT=wq[:, kt, p * P:(p + 1) * P],
                            rhs=xsl[:, kt, :],
                            start=(kt == 0), stop=(kt == KT - 1),
                        )
                    qts = qtsp.tile([P, 512], BF16, tag="qts")
                    psum_copy(qts, qt_ps)
                    qts_all.append(qts)
                for p in range(PAIRS):
                    out_ps = ps_out.tile([P, 512], F32, tag="psout")
                    nc.tensor.matmul(out_ps, lhsT=ctx_bd[:, p, :], rhs=qts_all[p],
                                     start=True, stop=True)
                    nc.vector.tensor_add(xp[:, p, :], out_ps, xsl[:, p, :])
                if spill_to is not None:
                    nc.sync.dma_start(
                        out=spill_to[:, :, ch * 512:(ch + 1) * 512], in_=xp)
                if xp_sink is not None:
                    xp_sink.append(xp)
                return xp

            def ctx_plus_ident(ctx_bd):
                """ctxI = ctx_bd + I so the out matmul computes q@ctx + x' in
                one shot (residual folded into the stationary operand)."""
                ctxI = ctxsb.tile([P, PAIRS, P], BF16, tag="ctx_bd")
                for p in range(PAIRS):
                    nc.vector.tensor_add(ctxI[:, p, :], ctx_bd[:, p, :], ident)
                return ctxI

            def cross_out_chunk(o_r, ctxI_bd, xpsl, ch):
                """o chunk = q@(ctx+I) = q@ctx + x' for 512 cols; all 8 pairs
                staged into one tile, written with a single batched DMA."""
                for h in range(2):
                    stg = outst.tile([P, 4, 512], BF16, tag="stg")
                    for i in range(4):
                        p = h * 4 + i
                        out_ps = ps_out.tile([P, 512], F32, tag="psout")
                        nc.tensor.matmul(out_ps, lhsT=ctxI_bd[:, p, :],
                                         rhs=xpsl[:, p, :], start=True, stop=True)
                        psum_copy(stg[:, i, :], out_ps)
                    nc.sync.dma_start(
                        out=o_r[:, h * 4:(h + 1) * 4, ch * 512:(ch + 1) * 512],
                        in_=stg)

            # ---- weight loads, interleaved with the first x slab so the
            # first kv GEMM starts after ~2 small DMAs (q part last) ----
            wk_s = wts.tile([P, KT, C], BF16, tag="wk")
            nc.sync.dma_start(out=wk_s[:, :, 0:512], in_=Wsq_r[:, :, C:C + 512])
            sl0 = xrot.tile([P, KT, 512], BF16, tag="xsl", name="x1_0")
            nc.sync.dma_start(out=sl0[:, 0:4, :], in_=x1T_r[:, 0:4, 0:512])
            nc.sync.dma_start(out=sl0[:, 4:8, :], in_=x1T_r[:, 4:8, 0:512])
            nc.sync.dma_start(out=wk_s[:, :, 512:C], in_=Wsq_r[:, :, C + 512:2 * C])
            x1sl = [sl0]
            wv_s = wts.tile([P, KT, C], BF16, tag="wv")
            nc.sync.dma_start(out=wv_s[:, :, 0:512], in_=Wsq_r[:, :, 2 * C:2 * C + 512])
            nc.sync.dma_start(out=wv_s[:, :, 512:C], in_=Wsq_r[:, :, 2 * C + 512:3 * C])
            x1sl += [load_x_slab(x1T_r, s, "x1") for s in range(1, CH)]
            wq = wts.tile([P, KT, C], BF16, tag="wq")
            nc.sync.dma_start(out=wq, in_=Wsq_r[:, :, 0:C])

            # ---- stream 1: self stage ----
            banks1 = ctx_accumulate(x1sl, wk_s, wv_s)
            ctx1s_bd = softmax_all(banks1)
            x2sl = []
            for ch in range(CH):
                self_out_chunk(x1sl[ch], wq, ctx1s_bd, ch, None, spill_to=x1p_r)
                x2sl.append(load_x_slab(x2T_r, ch, "x2"))

            # ---- stream 2: self stage (x2' stays resident) ----
            banks2 = ctx_accumulate(x2sl, wk_s, wv_s)
            ctx2s_bd = softmax_all(banks2)
            x2p = []
            for ch in range(CH):
                self_out_chunk(x2sl[ch], wq, ctx2s_bd, ch, x2p)

            # ---- cross: ctx2 from x2' ----
            wk2 = wts.tile([P, KT, C], BF16, tag="wk")
            nc.sync.dma_start(out=wk2, in_=Wkv2_r[:, :, 0:C])
            wv2 = wts.tile([P, KT, C], BF16, tag="wv")
            nc.sync.dma_start(out=wv2, in_=Wkv2_r[:, :, C:2 * C])
            # Wkv1's k-half reuses the wq slot (dead after S2c): its load
            # completes during C2a, so the fused phase starts without a
            # weight-load stall.
            wk1 = wts.tile([P, KT, C], BF16, tag="wq")
            nc.sync.dma_start(out=wk1, in_=Wkv1_r[:, :, 0:C])
            banksc2 = ctx_accumulate(x2p, wk2, wv2)
            ctx2_bd = ctx_plus_ident(softmax_all(banksc2))

            # ---- fused: ctx1' accumulation + o1 out (q = x1', streamed back).
            # kv GEMMs first in each chunk: they only need wk1/wv1 + the slab,
            # so they hide the softmax-c2 latency that gates the o1 outs.
            # wv1 waits for wk2's slot (C2a end); its first half arrives just
            # as the first v-chunk needs it. ----
            wv1 = wts.tile([P, KT, C], BF16, tag="wk")
            nc.sync.dma_start(out=wv1[:, :, 0:512], in_=Wkv1_r[:, :, C:C + 512])
            nc.sync.dma_start(out=wv1[:, :, 512:C], in_=Wkv1_r[:, :, C + 512:2 * C])
            banksc1 = ctx_banks()
            for ch in range(CH):
                x1p_sl = xrot.tile([P, KT, 512], BF16, tag="xsl", name=f"x1pr_{ch}")
                nc.sync.dma_start(out=x1p_sl,
                                  in_=x1p_r[:, :, ch * 512:(ch + 1) * 512])
                if ch == CH - 1:
                    # last chunk: o1 out first, so softmax-c1's DVE/ACT chains
                    # aren't queued behind o1's copy backlog
                    cross_out_chunk(o1T_r, ctx2_bd, x1p_sl, ch)
                for nt in range(ch * 4, ch * 4 + 4):
                    kv = kv_nt([None] * ch + [x1p_sl] + [None] * (CH - ch - 1),
                               wk1, wv1, nt)
                    pair_mms(banksc1, kv, nt)
                if ch < CH - 1:
                    cross_out_chunk(o1T_r, ctx2_bd, x1p_sl, ch)
            # ---- softmax-c1 fused per-pair with o2 out (pair p's outputs
            # start as soon as its own softmax is done; 4-deep gp PSUM) ----
            ctxI1 = ctxsb.tile([P, PAIRS, P], BF16, tag="ctx_bd")
            Ss = [softmax_chain(banksc1, p) for p in range(PAIRS)]
            for p in range(PAIRS):
                tr_ps = ps_out.tile([P, P], BF16, tag="psout")
                nc.tensor.transpose(tr_ps, Ss[p], ident)
                nc.vector.tensor_add(ctxI1[:, p, :], tr_ps, ident)
                for h in range(2):
                    stg = outst.tile([P, 4, 512], BF16, tag="stg")
                    for i in range(4):
                        ch = h * 4 + i
                        out_ps = ps_kv.tile([P, 512], F32, tag="gp")
                        nc.tensor.matmul(out_ps, lhsT=ctxI1[:, p, :],
                                         rhs=x2p[ch][:, p, :], start=True, stop=True)
                        psum_copy(stg[:, i, :], out_ps)
                    nc.sync.dma_start(
                        out=o2T_r[:, p, h * 2048:(h + 1) * 2048], in_=stg)

    nc.finalize()
    return nc


def _get_nc():
    if "nc" not in _CACHE:
        _CACHE["nc"] = _build()
    return _CACHE["nc"]


def kernel(x1, x2, Wsqkv1, Wkv1, Wkv2, num_heads=16, selfattn=1, **_unused):
    x1 = np.asarray(x1, dtype=np.float32)
    x2 = np.asarray(x2, dtype=np.float32)
    Wsq_b = np.ascontiguousarray(np.asarray(Wsqkv1, np.float32)).astype(ml_dtypes.bfloat16)
    Wkv1_b = np.ascontiguousarray(np.asarray(Wkv1, np.float32)).astype(ml_dtypes.bfloat16)
    Wkv2_b = np.ascontiguousarray(np.asarray(Wkv2, np.float32)).astype(ml_dtypes.bfloat16)

    nc = _get_nc()
    in_maps = []
    for b in range(B):
        in_maps.append({
            "x1T": np.ascontiguousarray(x1[b].T).astype(ml_dtypes.bfloat16),
            "x2T": np.ascontiguousarray(x2[b].T).astype(ml_dtypes.bfloat16),
            "Wsqkv": Wsq_b,
            "Wkv1": Wkv1_b,
            "Wkv2": Wkv2_b,
        })
    res = run_bass_kernel_spmd(nc, in_maps, core_ids=list(range(B)),
                               trace=bool(int(os.environ.get("KERNEL_TRACE", "0"))))
    _CACHE["last_result"] = res
    o1 = np.stack([np.asarray(res.results[b]["o1T"], np.float32).T for b in range(B)])
    o2 = np.stack([np.asarray(res.results[b]["o2T"], np.float32).T for b in range(B)])
    return o1, o2
